# revision 36
# baseline (speedup 1.0000x reference)
"""MoE FFN (top-2 routing, 8 experts) on 8 Trainium2 NeuronCores.

Strategy (expert parallelism, per the sharding hint):
  - Host computes router logits / top-2 / softmax (tiny: T x E) and
    dispatches tokens: expert e's tokens are gathered into a padded
    [H, C] batch for core e (C = common capacity).
  - Core e runs the dense FFN for its expert on its gathered tokens:
        yT = wt * ( GELU_tanh(x @ W1 + b1) @ W2 + b2 )^T
    computed fully transposed ([F,C] then [H,C]) so both matmuls use
    the weights as the stationary operand and no on-device transposes
    are needed. Matmul operands are fp16; accumulation is fp32 in
    PSUM; bias+GELU on the scalar engine, the per-token combine
    weight on the DVE (reading PSUM directly), output DMA'd as fp16.
  - C is trimmed to the actual max expert load (rounded to 8), split
    into column blocks of <=512 (PSUM bank width); the whole capacity
    is processed in ONE phase-A pass then ONE phase-B pass, so the
    ragged tail block costs only its real column count.
  - Host scatter-adds each core's [H, C] result back into [T, H].

Self-contained: hardcodes the problem shapes (H=768, F=3072, E=8, K=2).
"""

import os
import time

import numpy as np

H = 768
F = 3072
E = 8
K = 2
N_CORES = 8
P = 128
BANK = 512  # fp32 PSUM bank = 512 elems

PRECISION = os.environ.get("MOE_PRECISION", "fp16")  # "fp16" | "bf16" | "fp32"


def _blocks_of(C):
    out = []
    c0 = 0
    while c0 < C:
        w = min(BANK, C - c0)
        out.append((c0, w))
        c0 += w
    return out


# ---------------------------------------------------------------------------
# Bass/Tile device kernel
# ---------------------------------------------------------------------------

def _build_bass(C, use_b2, Hd=H, Fd=F, precision=None):
    """Build + compile the per-core Bass program for capacity C."""
    from contextlib import ExitStack

    import concourse.bass as bass  # noqa: F401
    import concourse.tile as tile
    from concourse import bacc, mybir
    from concourse._compat import with_exitstack

    precision = precision or PRECISION
    assert C % 4 == 0 and Hd % P == 0 and Fd % (8 * P) == 0
    FM = Fd // P          # number of 128-row tiles of the F dim
    HK = Hd // P          # contraction tiles for x@W1
    HN = Hd // P          # output row tiles of yT
    f32 = mybir.dt.float32
    mdt = {"bf16": mybir.dt.bfloat16, "fp16": mybir.dt.float16,
           "fp32": f32}[precision]

    blocks = _blocks_of(C)
    W0 = blocks[0][1]            # first block width (<= 512)
    CB = C - W0                  # columns in the xgb tile

    nc = bacc.Bacc("TRN2", target_bir_lowering=False, debug=False,
                   num_devices=N_CORES)
    # All inputs are pre-shuffled on the host into partition-major SBUF
    # layout so every DMA is a plain contiguous [128, N] copy (multi-KB
    # descriptor rows -> full ring bandwidth; the on-device `rearrange`
    # gathers ran at ~1/3 of peak).
    # x gathered+transposed, split so the first block's matmuls gate only
    # on the small leading piece.
    xga = nc.dram_tensor("xga", [P, HK * W0], mdt, kind="ExternalInput").ap()
    if CB:
        xgb = nc.dram_tensor("xgb", [P, HK * CB], mdt,
                             kind="ExternalInput").ap()
    # Uneven W1 pieces: a small first piece so the first matmul gates on
    # minimal data, then 3-fm-tile pieces streaming under the compute.
    PIECES = [1] * 6 + [2] * 3 + [3] * ((FM - 12) // 3)
    assert sum(PIECES) == FM
    w1d = [nc.dram_tensor(f"w1_{g}", [P, HK * n * P], mdt,
                          kind="ExternalInput").ap()
           for g, n in enumerate(PIECES)]
    W2G = FM // 2
    w2d = [nc.dram_tensor(f"w2_{g}", [P, W2G * Hd], mdt,
                          kind="ExternalInput").ap() for g in range(2)]
    # small fp32 constants: b1 [P, FM]; b2+combine weights [P, HN + C]
    b1d = nc.dram_tensor("b1d", [P, FM], f32, kind="ExternalInput").ap()
    cpk = nc.dram_tensor("cpk", [P, HN + C], f32, kind="ExternalInput").ap()
    y = nc.dram_tensor("y", [Hd, C], mdt, kind="ExternalOutput").ap()

    gelu = mybir.ActivationFunctionType.Gelu_apprx_tanh
    ident = mybir.ActivationFunctionType.Identity

    # fm -> (piece index, offset within piece)
    fm_loc = []
    for g, n in enumerate(PIECES):
        for j in range(n):
            fm_loc.append((g, j))

    @with_exitstack
    def body(ctx: ExitStack, tc: tile.TileContext):
        const = ctx.enter_context(tc.tile_pool(name="const", bufs=1))
        w1p = ctx.enter_context(tc.tile_pool(name="w1p", bufs=1))
        w2p = ctx.enter_context(tc.tile_pool(name="w2p", bufs=1))
        xp = ctx.enter_context(tc.tile_pool(name="xp", bufs=1))
        hp = ctx.enter_context(tc.tile_pool(name="hp", bufs=1))
        yp = ctx.enter_context(tc.tile_pool(name="yp", bufs=6))
        psAp = ctx.enter_context(tc.tile_pool(name="psA", bufs=4, space="PSUM"))
        psBp = ctx.enter_context(tc.tile_pool(name="psB", bufs=4, space="PSUM"))

        # --- DMA issue order == ring priority (each ring is FIFO). ---
        # Phase A consumes W1 at only ~70 GB/s, so the only hard ordering
        # constraint is x (all of it) + the first W1 piece early; cpk and
        # W2 are not needed until phase B (~half-way through the run).
        # Two DMA issue queues (each FIFO). The queues share the 16 DMA
        # engines with roughly fair scheduling, so late-need bulk must sit
        # BEHIND first-need data in the same FIFO — a third queue would
        # steal bandwidth from the critical path. The scalar queue's first
        # issue is also delayed ~1.5us by the preamble GELU table load, so
        # ring B rides the otherwise-idle GpSimd queue.
        # sync:   xga lo | w1 even | xgb lo | w2 half 0
        # gpsimd: xga hi | b1 | w1 odd | xgb hi | w2 half 1 | cpk
        # (phase A runs block-major, so xgb isn't needed until the block-0
        #  pass ends ~30us in, and W1 streams tile-by-tile under block 0.)
        ringb = nc.gpsimd
        xgat = xp.tile([P, HK * W0], mdt, tag="xga", name="xga")
        HH = HK // 2
        nc.sync.dma_start(xgat[:, 0:HH * W0], xga[:, 0:HH * W0])
        ringb.dma_start(xgat[:, HH * W0:], xga[:, HH * W0:])
        b1s = const.tile([P, FM], f32, name="b1s")
        ringb.dma_start(b1s[:], b1d[:, :])
        w1q = []
        for g, n in enumerate(PIECES):
            t = w1p.tile([P, HK * n * P], mdt, tag=f"w1q{g}", name=f"w1q{g}")
            w1q.append(t)
            (nc.sync if g % 2 == 0 else ringb).dma_start(t[:], w1d[g][:, :])
        xgbt = None
        if CB:
            xgbt = xp.tile([P, HK * CB], mdt, tag="xgb", name="xgb")
            nc.sync.dma_start(xgbt[:, 0:HH * CB], xgb[:, 0:HH * CB])
            ringb.dma_start(xgbt[:, HH * CB:], xgb[:, HH * CB:])
        w2g = []
        for g in range(2):
            t = w2p.tile([P, W2G * Hd], mdt, tag=f"w2g{g}", name=f"w2g{g}")
            (nc.sync if g % 2 == 0 else ringb).dma_start(t[:], w2d[g][:, :])
            w2g.append(t)
        w2t = [w2g[k // W2G][:, (k % W2G) * Hd:(k % W2G + 1) * Hd]
               for k in range(FM)]
        cps2 = const.tile([P, HN + C], f32, name="cps2")
        b2s = cps2[:, 0:HN]
        wtbs = cps2[:, HN:]
        ringb.dma_start(cps2[:], cpk[:, :])

        def w1_tile(hk, fm):
            g, j = fm_loc[fm]
            fw = PIECES[g] * P
            return w1q[g][:, hk * fw + j * P:hk * fw + (j + 1) * P]

        def rhs_x(hk, c0, w):
            if c0 < W0:
                return xgat[:, hk * W0 + c0:hk * W0 + c0 + w]
            cb = c0 - W0
            return xgbt[:, hk * CB + cb:hk * CB + cb + w]

        # Pre-warm the PE clock during the DMA-bound startup: dummy
        # matmuls on a memset tile keep the HAM activity monitor fed so
        # the p-state ramp completes before the real data lands (~6 us).
        wtile = xp.tile([P, 256], mdt, tag="warm", name="warm")
        nc.vector.memset(wtile[:], 0.0)
        wps = psBp.tile([P, BANK], f32, tag="psB", name="warmps")
        NWARM = 40
        for i in range(NWARM):
            nc.tensor.matmul(wps[:, 0:256], lhsT=wtile[:, 0:P], rhs=wtile[:],
                             start=(i == 0), stop=(i == NWARM - 1))

        # ---- phase A: hT[f, c] = gelu((x@W1)[c, f] + b1[f]) ----
        # Block-major: the whole block-0 pass needs only xga + streaming
        # W1, so compute starts as soon as the first W1 piece lands, and a
        # slow ring shows up as many short stalls (below the HAM throttle
        # threshold) rather than one long one.
        hts = [hp.tile([P, C], mdt, tag=f"hts{fm}", name=f"hts{fm}")
               for fm in range(FM)]
        # The ragged tail block rides in the same per-fm pass as block 1 so
        # its tiny activations stay hidden under the 512-wide matmuls.
        groups = [blocks[:1]] + [blocks[i:i + 2]
                                 for i in range(1, len(blocks), 2)]
        for grp in groups:
            for fm in range(FM):
                pss = []
                for c0, w in grp:
                    ps = psAp.tile([P, BANK], f32, tag="psA", name="psA")
                    pss.append(ps)
                    for hk in range(HK):
                        nc.tensor.matmul(
                            ps[:, :w],
                            lhsT=w1_tile(hk, fm),
                            rhs=rhs_x(hk, c0, w),
                            start=(hk == 0), stop=(hk == HK - 1),
                        )
                for ps, (c0, w) in zip(pss, grp):
                    nc.scalar.activation(hts[fm][:, c0:c0 + w], ps[:, :w],
                                         gelu, bias=b1s[:, fm:fm + 1])

        # ---- phase B: yT[h, c] = wt[c] * (sum_f W2[f, h] * hT[f, c] + b2) --
        for hn in range(HN):
            for bi, (c0, w) in enumerate(blocks):
                ps = psBp.tile([P, BANK], f32, tag="psB", name="psB")
                for fk in range(FM):
                    nc.tensor.matmul(
                        ps[:, :w],
                        lhsT=w2t[fk][:, hn * P:(hn + 1) * P],
                        rhs=hts[fk][:, c0:c0 + w],
                        start=(fk == 0), stop=(fk == FM - 1),
                    )
                ot = yp.tile([P, BANK], mdt, tag="yout", name="yout")
                if use_b2:
                    ob = yp.tile([P, BANK], f32, tag="yb", name="yb")
                    nc.scalar.activation(ob[:, :w], ps[:, :w], ident,
                                         bias=b2s[:, hn:hn + 1])
                    nc.vector.tensor_mul(ot[:, :w], ob[:, :w],
                                         wtbs[:, c0:c0 + w])
                else:
                    nc.vector.tensor_mul(ot[:, :w], ps[:, :w],
                                         wtbs[:, c0:c0 + w])
                (nc.sync if bi % 2 == 0 else nc.scalar).dma_start(
                    y[hn * P:(hn + 1) * P, c0:c0 + w], ot[:, :w])

    with tile.TileContext(nc) as tc:
        body(tc)
    nc.compile()
    return nc


# ---------------------------------------------------------------------------
# Host-side routing + dispatch
# ---------------------------------------------------------------------------

def _route(xf, gate_w):
    """Top-2 router in float64 for a numerically robust top-k set.

    Returns per-expert (token_idx, weight) lists.
    """
    logits = xf.astype(np.float64) @ gate_w.astype(np.float64)  # [T, E]
    top_idx = np.argpartition(logits, E - K, axis=1)[:, E - K:]  # [T, K]
    top_val = np.take_along_axis(logits, top_idx, axis=1)
    m = top_val.max(axis=1, keepdims=True)
    ex = np.exp(top_val - m)
    wts = ex / ex.sum(axis=1, keepdims=True)  # [T, K] float64

    toks, ws = [], []
    for e in range(E):
        mask = top_idx == e  # [T, K]
        rows = np.nonzero(mask.any(axis=1))[0]
        toks.append(rows)
        ws.append(wts[mask].astype(np.float32))
    return toks, ws


def _np_mdt():
    import ml_dtypes
    return {"bf16": ml_dtypes.bfloat16, "fp16": np.float16,
            "fp32": np.float32}[PRECISION]


def _make_in_maps(xf, gate_w, W1, b1, W2, b2):
    toks, ws = _route(xf, gate_w)
    nmax = max(len(t) for t in toks)
    C = max(P, ((nmax + 3) // 4) * 4)
    W0 = min(BANK, C)
    mdt = _np_mdt()

    W1a = np.asarray(W1, np.float32)
    b1a = np.asarray(b1, np.float32)
    W2a = np.asarray(W2, np.float32)
    b2a = np.asarray(b2, np.float32)
    HK = H // P
    FM = F // P
    PIECES = [1] * 6 + [2] * 3 + [3] * ((FM - 12) // 3)
    W2G = FM // 2
    in_maps = []
    for e in range(E):
        n_e = len(toks[e])
        xgT = np.zeros((H, C), mdt)
        xgT[:, :n_e] = xf[toks[e]].T.astype(mdt)
        # partition-major: [P, HK, C] so every DMA row is contiguous
        xgP = np.ascontiguousarray(xgT.reshape(HK, P, C).transpose(1, 0, 2))
        w1P = W1a[e].astype(mdt).reshape(HK, P, F).transpose(1, 0, 2)
        w2P = W2a[e].astype(mdt).reshape(FM, P, H).transpose(1, 0, 2)
        wtb = np.zeros((P, C), np.float32)
        wtb[:, :n_e] = ws[e][None, :]
        cpk = np.concatenate([
            b2a[e].reshape(H // P, P).T,
            wtb,
        ], axis=1)
        m = {
            "xga": np.ascontiguousarray(xgP[:, :, :W0]).reshape(P, -1),
            "b1d": np.ascontiguousarray(b1a[e].reshape(F // P, P).T),
            "cpk": np.ascontiguousarray(cpk),
        }
        f0 = 0
        for g, n in enumerate(PIECES):
            m[f"w1_{g}"] = np.ascontiguousarray(
                w1P[:, :, f0:f0 + n * P]).reshape(P, -1)
            f0 += n * P
        for g in range(2):
            m[f"w2_{g}"] = np.ascontiguousarray(
                w2P[:, g * W2G:(g + 1) * W2G, :]).reshape(P, -1)
        if C > W0:
            m["xgb"] = np.ascontiguousarray(xgP[:, :, W0:]).reshape(P, -1)
        in_maps.append(m)
    return in_maps, toks, C


def _run(inputs, trace=False):
    global PRECISION
    from concourse.bass_utils import run_bass_kernel_spmd

    x, gate_w, W1, b1, W2, b2 = (inputs[k] for k in
                                 ("x", "gate_w", "W1", "b1", "W2", "b2"))
    x = np.asarray(x)
    Bb, S, Hd = x.shape
    assert Hd == H
    T = Bb * S
    xf = np.ascontiguousarray(x.reshape(T, Hd), dtype=np.float32)
    gate_w = np.asarray(gate_w, np.float32)

    # fp16 matmul operands need moderate dynamic range; fall back to
    # bf16 (full fp32 exponent range) if the data is far outside the
    # expected unit-scale regime.
    if PRECISION == "fp16":
        amax = max(float(np.abs(np.asarray(t)).max())
                   for t in (xf, W1, W2))
        if not np.isfinite(amax) or amax > 1e3:
            PRECISION = "bf16"

    use_b2 = bool(np.any(np.asarray(b2)))
    in_maps, toks, C = _make_in_maps(xf, gate_w, W1, b1, W2, b2)
    nc = _build_bass(C, use_b2)

    kwargs = {}
    if trace:
        kwargs = dict(trace=True, trace_cores=list(range(N_CORES)))
    try:
        res = run_bass_kernel_spmd(nc, in_maps, core_ids=list(range(N_CORES)),
                                   **kwargs)
    except Exception:
        # One retry for transient device faults.
        time.sleep(5)
        res = run_bass_kernel_spmd(nc, in_maps, core_ids=list(range(N_CORES)),
                                   **kwargs)
    out = np.zeros((T, H), np.float32)
    for e in range(E):
        n_e = len(toks[e])
        out[toks[e]] += res.results[e]["y"][:, :n_e].T.astype(np.float32)
    return out.reshape(Bb, S, Hd), res


def kernel(x, gate_w, W1, b1, W2, b2):
    out, _ = _run({"x": x, "gate_w": gate_w, "W1": W1, "b1": b1,
                   "W2": W2, "b2": b2})
    return out.astype(np.asarray(x).dtype, copy=False)


# Exposed for test.py: run with profiling, return (output, BassKernelResults)
def kernel_profiled(x, gate_w, W1, b1, W2, b2):
    return _run({"x": x, "gate_w": gate_w, "W1": W1, "b1": b1,
                 "W2": W2, "b2": b2}, trace=True)


# revision 37
# speedup vs baseline: 1.1798x; 1.1798x over previous
"""MoE FFN (top-2 routing, 8 experts) on 8 Trainium2 NeuronCores.

Strategy (expert parallelism, per the sharding hint):
  - Host computes router logits / top-2 / softmax (tiny: T x E) and
    dispatches tokens: expert e's tokens are gathered into a padded
    [H, C] batch for core e (C = common capacity).
  - Core e runs the dense FFN for its expert on its gathered tokens:
        yT = wt * ( GELU_tanh(x @ W1 + b1) @ W2 + b2 )^T
    computed fully transposed ([F,C] then [H,C]) so both matmuls use
    the weights as the stationary operand and no on-device transposes
    are needed. Matmul operands are fp16; accumulation is fp32 in
    PSUM; bias+GELU on the scalar engine, the per-token combine
    weight on the DVE (reading PSUM directly), output DMA'd as fp16.
  - C is trimmed to the actual max expert load (rounded to 8), split
    into column blocks of <=512 (PSUM bank width); the whole capacity
    is processed in ONE phase-A pass then ONE phase-B pass, so the
    ragged tail block costs only its real column count.
  - Host scatter-adds each core's [H, C] result back into [T, H].

Self-contained: hardcodes the problem shapes (H=768, F=3072, E=8, K=2).
"""

import os
import time

import numpy as np

H = 768
F = 3072
E = 8
K = 2
N_CORES = 8
P = 128
BANK = 512  # fp32 PSUM bank = 512 elems

PRECISION = os.environ.get("MOE_PRECISION", "fp16")  # "fp16" | "bf16" | "fp32"


def _blocks_of(C):
    out = []
    c0 = 0
    while c0 < C:
        w = min(BANK, C - c0)
        out.append((c0, w))
        c0 += w
    return out


# ---------------------------------------------------------------------------
# Bass/Tile device kernel
# ---------------------------------------------------------------------------

def _build_bass(C, use_b2, Hd=H, Fd=F, precision=None):
    """Build + compile the per-core Bass program for capacity C."""
    from contextlib import ExitStack

    import concourse.bass as bass  # noqa: F401
    import concourse.tile as tile
    from concourse import bacc, mybir
    from concourse._compat import with_exitstack

    precision = precision or PRECISION
    assert C % 4 == 0 and Hd % P == 0 and Fd % (8 * P) == 0
    FM = Fd // P          # number of 128-row tiles of the F dim
    HK = Hd // P          # contraction tiles for x@W1
    HN = Hd // P          # output row tiles of yT
    f32 = mybir.dt.float32
    mdt = {"bf16": mybir.dt.bfloat16, "fp16": mybir.dt.float16,
           "fp32": f32}[precision]

    blocks = _blocks_of(C)
    W0 = blocks[0][1]            # first block width (<= 512)
    CB = C - W0                  # columns in the xgb tile

    nc = bacc.Bacc("TRN2", target_bir_lowering=False, debug=False,
                   num_devices=N_CORES)
    # All inputs are pre-shuffled on the host into partition-major SBUF
    # layout so every DMA is a plain contiguous [128, N] copy (multi-KB
    # descriptor rows -> full ring bandwidth; the on-device `rearrange`
    # gathers ran at ~1/3 of peak).
    # x gathered+transposed, split so the first block's matmuls gate only
    # on the small leading piece.
    xga = nc.dram_tensor("xga", [P, HK * W0], mdt, kind="ExternalInput").ap()
    if CB:
        xgb = nc.dram_tensor("xgb", [P, HK * CB], mdt,
                             kind="ExternalInput").ap()
    # Uneven W1 pieces: a small first piece so the first matmul gates on
    # minimal data, then 3-fm-tile pieces streaming under the compute.
    PIECES = [1, 1, 2, 2] + [3] * ((FM - 6) // 3)
    assert sum(PIECES) == FM
    w1d = [nc.dram_tensor(f"w1_{g}", [P, HK * n * P], mdt,
                          kind="ExternalInput").ap()
           for g, n in enumerate(PIECES)]
    W2G = FM // 2
    w2d = [nc.dram_tensor(f"w2_{g}", [P, W2G * Hd], mdt,
                          kind="ExternalInput").ap() for g in range(2)]
    # small fp32 constants: b1 [P, FM]; b2+combine weights [P, HN + C]
    b1d = nc.dram_tensor("b1d", [P, FM], f32, kind="ExternalInput").ap()
    cpk = nc.dram_tensor("cpk", [P, HN + C], f32, kind="ExternalInput").ap()
    y = nc.dram_tensor("y", [Hd, C], mdt, kind="ExternalOutput").ap()

    gelu = mybir.ActivationFunctionType.Gelu_apprx_tanh
    ident = mybir.ActivationFunctionType.Identity

    # fm -> (piece index, offset within piece)
    fm_loc = []
    for g, n in enumerate(PIECES):
        for j in range(n):
            fm_loc.append((g, j))

    @with_exitstack
    def body(ctx: ExitStack, tc: tile.TileContext):
        const = ctx.enter_context(tc.tile_pool(name="const", bufs=1))
        w1p = ctx.enter_context(tc.tile_pool(name="w1p", bufs=1))
        w2p = ctx.enter_context(tc.tile_pool(name="w2p", bufs=1))
        xp = ctx.enter_context(tc.tile_pool(name="xp", bufs=1))
        hp = ctx.enter_context(tc.tile_pool(name="hp", bufs=1))
        yp = ctx.enter_context(tc.tile_pool(name="yp", bufs=6))
        psAp = ctx.enter_context(tc.tile_pool(name="psA", bufs=4, space="PSUM"))
        psBp = ctx.enter_context(tc.tile_pool(name="psB", bufs=4, space="PSUM"))

        # --- DMA issue order == ring priority (each ring is FIFO). ---
        # Phase A consumes W1 at only ~70 GB/s, so the only hard ordering
        # constraint is x (all of it) + the first W1 piece early; cpk and
        # W2 are not needed until phase B (~half-way through the run).
        # Two DMA issue queues (each FIFO). The queues share the 16 DMA
        # engines with roughly fair scheduling, so late-need bulk must sit
        # BEHIND first-need data in the same FIFO — a third queue would
        # steal bandwidth from the critical path. The scalar queue's first
        # issue is also delayed ~1.5us by the preamble GELU table load, so
        # ring B rides the otherwise-idle GpSimd queue.
        # sync:   xga lo | w1 even | xgb lo | w2 half 0
        # gpsimd: xga hi | b1 | w1 odd | xgb hi | w2 half 1 | cpk
        # (phase A runs block-major, so xgb isn't needed until the block-0
        #  pass ends ~30us in, and W1 streams tile-by-tile under block 0.)
        ringb = nc.gpsimd
        xgat = xp.tile([P, HK * W0], mdt, tag="xga", name="xga")
        HH = HK // 2
        nc.sync.dma_start(xgat[:, 0:HH * W0], xga[:, 0:HH * W0])
        ringb.dma_start(xgat[:, HH * W0:], xga[:, HH * W0:])
        b1s = const.tile([P, FM], f32, name="b1s")
        ringb.dma_start(b1s[:], b1d[:, :])
        w1q = []
        for g, n in enumerate(PIECES):
            t = w1p.tile([P, HK * n * P], mdt, tag=f"w1q{g}", name=f"w1q{g}")
            w1q.append(t)
            (nc.sync if g % 2 == 0 else ringb).dma_start(t[:], w1d[g][:, :])
        xgbt = None
        if CB:
            xgbt = xp.tile([P, HK * CB], mdt, tag="xgb", name="xgb")
            nc.sync.dma_start(xgbt[:, 0:HH * CB], xgb[:, 0:HH * CB])
            ringb.dma_start(xgbt[:, HH * CB:], xgb[:, HH * CB:])
        w2g = []
        for g in range(2):
            t = w2p.tile([P, W2G * Hd], mdt, tag=f"w2g{g}", name=f"w2g{g}")
            (nc.sync if g % 2 == 0 else ringb).dma_start(t[:], w2d[g][:, :])
            w2g.append(t)
        w2t = [w2g[k // W2G][:, (k % W2G) * Hd:(k % W2G + 1) * Hd]
               for k in range(FM)]
        cps2 = const.tile([P, HN + C], f32, name="cps2")
        b2s = cps2[:, 0:HN]
        wtbs = cps2[:, HN:]
        ringb.dma_start(cps2[:], cpk[:, :])

        def w1_tile(hk, fm):
            g, j = fm_loc[fm]
            fw = PIECES[g] * P
            return w1q[g][:, hk * fw + j * P:hk * fw + (j + 1) * P]

        def rhs_x(hk, c0, w):
            if c0 < W0:
                return xgat[:, hk * W0 + c0:hk * W0 + c0 + w]
            cb = c0 - W0
            return xgbt[:, hk * CB + cb:hk * CB + cb + w]

        # Pre-warm the PE clock during the DMA-bound startup: dummy
        # matmuls on a memset tile keep the HAM activity monitor fed so
        # the p-state ramp completes before the real data lands (~6 us).
        wtile = xp.tile([P, 256], mdt, tag="warm", name="warm")
        nc.vector.memset(wtile[:], 0.0)
        wps = psBp.tile([P, BANK], f32, tag="psB", name="warmps")
        NWARM = 34
        for i in range(NWARM):
            nc.tensor.matmul(wps[:, 0:256], lhsT=wtile[:, 0:P], rhs=wtile[:],
                             start=(i == 0), stop=(i == NWARM - 1))

        # ---- phase A: hT[f, c] = gelu((x@W1)[c, f] + b1[f]) ----
        # Block-major: the whole block-0 pass needs only xga + streaming
        # W1, so compute starts as soon as the first W1 piece lands, and a
        # slow ring shows up as many short stalls (below the HAM throttle
        # threshold) rather than one long one.
        hts = [hp.tile([P, C], mdt, tag=f"hts{fm}", name=f"hts{fm}")
               for fm in range(FM)]
        # The ragged tail block rides in the same per-fm pass as block 1 so
        # its tiny activations stay hidden under the 512-wide matmuls.
        groups = [blocks[:1]] + [blocks[i:i + 2]
                                 for i in range(1, len(blocks), 2)]
        for grp in groups:
            for fm in range(FM):
                pss = []
                for c0, w in grp:
                    ps = psAp.tile([P, BANK], f32, tag="psA", name="psA")
                    pss.append(ps)
                    for hk in range(HK):
                        nc.tensor.matmul(
                            ps[:, :w],
                            lhsT=w1_tile(hk, fm),
                            rhs=rhs_x(hk, c0, w),
                            start=(hk == 0), stop=(hk == HK - 1),
                        )
                for ps, (c0, w) in zip(pss, grp):
                    nc.scalar.activation(hts[fm][:, c0:c0 + w], ps[:, :w],
                                         gelu, bias=b1s[:, fm:fm + 1])

        # ---- phase B: yT[h, c] = wt[c] * (sum_f W2[f, h] * hT[f, c] + b2) --
        for hn in range(HN):
            for bi, (c0, w) in enumerate(blocks):
                ps = psBp.tile([P, BANK], f32, tag="psB", name="psB")
                for fk in range(FM):
                    nc.tensor.matmul(
                        ps[:, :w],
                        lhsT=w2t[fk][:, hn * P:(hn + 1) * P],
                        rhs=hts[fk][:, c0:c0 + w],
                        start=(fk == 0), stop=(fk == FM - 1),
                    )
                ot = yp.tile([P, BANK], mdt, tag="yout", name="yout")
                if use_b2:
                    ob = yp.tile([P, BANK], f32, tag="yb", name="yb")
                    nc.scalar.activation(ob[:, :w], ps[:, :w], ident,
                                         bias=b2s[:, hn:hn + 1])
                    nc.vector.tensor_mul(ot[:, :w], ob[:, :w],
                                         wtbs[:, c0:c0 + w])
                else:
                    nc.vector.tensor_mul(ot[:, :w], ps[:, :w],
                                         wtbs[:, c0:c0 + w])
                (nc.sync if bi % 2 == 0 else nc.scalar).dma_start(
                    y[hn * P:(hn + 1) * P, c0:c0 + w], ot[:, :w])

    with tile.TileContext(nc) as tc:
        body(tc)
    nc.compile()
    return nc


# ---------------------------------------------------------------------------
# Host-side routing + dispatch
# ---------------------------------------------------------------------------

def _route(xf, gate_w):
    """Top-2 router in float64 for a numerically robust top-k set.

    Returns per-expert (token_idx, weight) lists.
    """
    logits = xf.astype(np.float64) @ gate_w.astype(np.float64)  # [T, E]
    top_idx = np.argpartition(logits, E - K, axis=1)[:, E - K:]  # [T, K]
    top_val = np.take_along_axis(logits, top_idx, axis=1)
    m = top_val.max(axis=1, keepdims=True)
    ex = np.exp(top_val - m)
    wts = ex / ex.sum(axis=1, keepdims=True)  # [T, K] float64

    toks, ws = [], []
    for e in range(E):
        mask = top_idx == e  # [T, K]
        rows = np.nonzero(mask.any(axis=1))[0]
        toks.append(rows)
        ws.append(wts[mask].astype(np.float32))
    return toks, ws


def _np_mdt():
    import ml_dtypes
    return {"bf16": ml_dtypes.bfloat16, "fp16": np.float16,
            "fp32": np.float32}[PRECISION]


def _make_in_maps(xf, gate_w, W1, b1, W2, b2):
    toks, ws = _route(xf, gate_w)
    nmax = max(len(t) for t in toks)
    C = max(P, ((nmax + 3) // 4) * 4)
    W0 = min(BANK, C)
    mdt = _np_mdt()

    W1a = np.asarray(W1, np.float32)
    b1a = np.asarray(b1, np.float32)
    W2a = np.asarray(W2, np.float32)
    b2a = np.asarray(b2, np.float32)
    HK = H // P
    FM = F // P
    PIECES = [1, 1, 2, 2] + [3] * ((FM - 6) // 3)
    W2G = FM // 2
    in_maps = []
    for e in range(E):
        n_e = len(toks[e])
        xgT = np.zeros((H, C), mdt)
        xgT[:, :n_e] = xf[toks[e]].T.astype(mdt)
        # partition-major: [P, HK, C] so every DMA row is contiguous
        xgP = np.ascontiguousarray(xgT.reshape(HK, P, C).transpose(1, 0, 2))
        w1P = W1a[e].astype(mdt).reshape(HK, P, F).transpose(1, 0, 2)
        w2P = W2a[e].astype(mdt).reshape(FM, P, H).transpose(1, 0, 2)
        wtb = np.zeros((P, C), np.float32)
        wtb[:, :n_e] = ws[e][None, :]
        cpk = np.concatenate([
            b2a[e].reshape(H // P, P).T,
            wtb,
        ], axis=1)
        m = {
            "xga": np.ascontiguousarray(xgP[:, :, :W0]).reshape(P, -1),
            "b1d": np.ascontiguousarray(b1a[e].reshape(F // P, P).T),
            "cpk": np.ascontiguousarray(cpk),
        }
        f0 = 0
        for g, n in enumerate(PIECES):
            m[f"w1_{g}"] = np.ascontiguousarray(
                w1P[:, :, f0:f0 + n * P]).reshape(P, -1)
            f0 += n * P
        for g in range(2):
            m[f"w2_{g}"] = np.ascontiguousarray(
                w2P[:, g * W2G:(g + 1) * W2G, :]).reshape(P, -1)
        if C > W0:
            m["xgb"] = np.ascontiguousarray(xgP[:, :, W0:]).reshape(P, -1)
        in_maps.append(m)
    return in_maps, toks, C


def _run(inputs, trace=False):
    global PRECISION
    from concourse.bass_utils import run_bass_kernel_spmd

    x, gate_w, W1, b1, W2, b2 = (inputs[k] for k in
                                 ("x", "gate_w", "W1", "b1", "W2", "b2"))
    x = np.asarray(x)
    Bb, S, Hd = x.shape
    assert Hd == H
    T = Bb * S
    xf = np.ascontiguousarray(x.reshape(T, Hd), dtype=np.float32)
    gate_w = np.asarray(gate_w, np.float32)

    # fp16 matmul operands need moderate dynamic range; fall back to
    # bf16 (full fp32 exponent range) if the data is far outside the
    # expected unit-scale regime.
    if PRECISION == "fp16":
        amax = max(float(np.abs(np.asarray(t)).max())
                   for t in (xf, W1, W2))
        if not np.isfinite(amax) or amax > 1e3:
            PRECISION = "bf16"

    use_b2 = bool(np.any(np.asarray(b2)))
    in_maps, toks, C = _make_in_maps(xf, gate_w, W1, b1, W2, b2)
    nc = _build_bass(C, use_b2)

    kwargs = {}
    if trace:
        kwargs = dict(trace=True, trace_cores=list(range(N_CORES)))
    try:
        res = run_bass_kernel_spmd(nc, in_maps, core_ids=list(range(N_CORES)),
                                   **kwargs)
    except Exception:
        # One retry for transient device faults.
        time.sleep(5)
        res = run_bass_kernel_spmd(nc, in_maps, core_ids=list(range(N_CORES)),
                                   **kwargs)
    out = np.zeros((T, H), np.float32)
    for e in range(E):
        n_e = len(toks[e])
        out[toks[e]] += res.results[e]["y"][:, :n_e].T.astype(np.float32)
    return out.reshape(Bb, S, Hd), res


def kernel(x, gate_w, W1, b1, W2, b2):
    out, _ = _run({"x": x, "gate_w": gate_w, "W1": W1, "b1": b1,
                   "W2": W2, "b2": b2})
    return out.astype(np.asarray(x).dtype, copy=False)


# Exposed for test.py: run with profiling, return (output, BassKernelResults)
def kernel_profiled(x, gate_w, W1, b1, W2, b2):
    return _run({"x": x, "gate_w": gate_w, "W1": W1, "b1": b1,
                 "W2": W2, "b2": b2}, trace=True)
